# revision 31
# baseline (speedup 1.0000x reference)
"""Trainium2 Bass kernel for CorrLayerDownsample.

Math (reference): hatx = fft2(xpsi); per-moment p: corr = ifft2(h1 * conj(h2)).real,
masked by masks_shift[shifted[p]], keep union_idx positions.

Device algorithm (per core), bf16 matmul pipeline with fp32 PSUM accumulation:
  - 2D DFT of the needed maps by PE matmuls, keeping only u = 0..64 rows
    (real-input Hermitian symmetry; row u>=65 of the spectrum is recovered in
    stage-2 via a x2 fold). T1 is one matmul per map (the map is the stationary
    operand, [FmRe|FmIm] the 130-wide moving operand); T2 uses the constant DFT
    matrices as stationary and batches 6 maps per matmul (390-wide moving).
  - Per moment, three Karatsuba product planes (DVE/GPSIMD bf16 ops over runs of
    moments sharing the m1 map); the complex combine happens inside the PSUM
    accumulation group:
      T[u, {re|im}] = sum_v P[v,u] * Wn[v, {yd}]       (stage 1, 3 matmuls/row)
      out[xd, (row, yd)] = sum_u WmRe[u,xd] T_re[u,row,yd]
                               + WmImNeg[u,xd] T_im[u,row,yd]
    Stage 2 is flipped: the fold constants are stationary and 30 rows ride in
    one 510-wide moving operand -> 2 matmuls per 30 rows instead of 2 per row.
  - The shift mask is separable and applied on the host during unshard; the
    device output is bf16 to halve the download.

Sharding: 8 cores = batch b (4) x moment parity (2). The moment pair pattern is
identical for the two parities (a2 is the innermost index-generation loop), so
a single SPMD program works: per-core inputs carry b's maps and the parity's
m2-side map subset.
"""

import sys

sys.path.insert(0, "/opt/trn_rl_repo")

import numpy as np
import ml_dtypes

BF16 = ml_dtypes.bfloat16

J, B, C, M, N = 4, 4, 8, 128, 128
UH = M // 2 + 1  # 65 kept u rows
NCORES = 8

_CACHE = {}


def _host_prep(la1, la2, shifted, union_idx, masks_shift):
    """Index analysis. Returns None if the fast-path assumptions fail."""
    P = la1.shape[0]
    if P % 2 != 0:
        return None
    m1 = la1[:, 0].astype(np.int64) * C + la1[:, 1]
    m2 = la2[:, 0].astype(np.int64) * C + la2[:, 1]
    if (m1 < 0).any() or (m1 >= J * C).any() or (m2 < 0).any() or (m2 >= J * C).any():
        return None
    xs, ys = union_idx // N, union_idx % N
    X, Y = np.unique(xs), np.unique(ys)
    NX, NY = len(X), len(Y)
    if NX * NY != len(union_idx) or NX > 64 or NY > 64:
        return None
    gx, gy = np.meshgrid(X, Y, indexing="ij")
    if not np.array_equal(np.sort(union_idx), np.sort((gx * N + gy).ravel())):
        return None
    # union_idx must be sorted x-major for the final scatter to be a transpose
    if not np.array_equal(union_idx, (gx * N + gy).ravel()):
        return None
    pe, po = np.arange(0, P, 2), np.arange(1, P, 2)
    if not np.array_equal(m1[pe], m1[po]):
        return None
    sub_e, sub_o = np.unique(m2[pe]), np.unique(m2[po])
    if len(sub_e) > 16 or len(sub_o) > 16 or len(sub_e) != len(sub_o):
        return None
    slot_e = np.searchsorted(sub_e, m2[pe])
    slot_o = np.searchsorted(sub_o, m2[po])
    if not np.array_equal(slot_e, slot_o):
        return None
    if not np.array_equal(shifted[pe], shifted[po]):
        return None
    order = np.lexsort((slot_e, m1[pe]))  # sorted row order, same for both halves
    m1_s, slot_s = m1[pe][order], slot_e[order]
    runs = []  # (m1, slot0, count)
    i = 0
    while i < len(m1_s):
        j = i
        while (
            j < len(m1_s)
            and m1_s[j] == m1_s[i]
            and slot_s[j] == slot_s[i] + (j - i)
        ):
            j += 1
        runs.append((int(m1_s[i]), int(slot_s[i]), j - i))
        i = j
    if len(runs) > 64:
        return None
    # Move up to KPOOL runs whose m1 map is in the first m1 FFT group to the
    # END of the row order and mark them for the GPSIMD engine: their inputs
    # are ready first, so GPSIMD computes the tail rows concurrently while
    # DVE streams through the bulk, and the drain is never GPSIMD-gated.
    KPOOL = 6
    early = [i for i, (a, _, _) in enumerate(runs) if a < 6][:KPOOL]
    if len(early) >= 2:
        keep = [i for i in range(len(runs)) if i not in early[1:]]
        new_idx = keep + early[1:]   # leave one early run for DVE's warmup
    else:
        new_idx = list(range(len(runs)))
    npool = len(early) - 1 if len(early) >= 2 else 0
    starts = np.concatenate([[0], np.cumsum([r[2] for r in runs])]).astype(int)
    perm = np.concatenate([np.arange(starts[i], starts[i + 1])
                           for i in new_idx]).astype(int)
    order = order[perm]
    runs = [runs[i] for i in new_idx]
    # per-row shift class (device row order) + per-class Y window: columns of
    # the union grid where that class's mask is nonzero. Stage 1/2 and the
    # download only carry those columns; the host scatters them back (and
    # multiplies by the actual mask values, so non-binary masks stay correct).
    row_cls = shifted[pe[order]].astype(np.int64)
    classes = {}
    for k in sorted(set(int(c) for c in row_cls)):
        Mk = masks_shift[k][np.ix_(X, Y)]
        Yi = np.where(np.abs(Mk).sum(axis=0) > 0)[0]
        if len(Yi) == 0:
            Yi = np.array([0])
        classes[k] = Yi.astype(np.int64)
    return dict(
        m1=m1, m2=m2, X=X, Y=Y, NX=NX, NY=NY, pe=pe, po=po,
        sub_e=sub_e, sub_o=sub_o, order=order, runs=runs, npool=npool,
        n_rows=len(order), nsub=len(sub_e), row_cls=row_cls, classes=classes,
    )


def _consts(prep):
    X, Y = prep["X"], prep["Y"]
    k = np.arange(M)
    th = 2 * np.pi * np.outer(k, k[:UH]) / M
    FmRe = np.cos(th).astype(np.float32)          # [m, u]
    FmIm = (-np.sin(th)).astype(np.float32)
    thn = 2 * np.pi * np.outer(k, k) / N
    FnRe = np.cos(thn).astype(np.float32)         # [n, v]
    FnIm = (-np.sin(thn)).astype(np.float32)
    cu = np.full(UH, 2.0, np.float32)
    cu[0] = 1.0
    if M % 2 == 0:
        cu[UH - 1] = 1.0
    thm = 2 * np.pi * np.outer(np.arange(UH), X) / M
    WmRe = (cu[:, None] * np.cos(thm) / M).astype(np.float32)      # [65, NX]
    WmImNeg = (-cu[:, None] * np.sin(thm) / M).astype(np.float32)  # [65, NX]
    # Karatsuba 3-mult complex product: with m1=h1r*h2r, m2=h1i*h2i,
    # m3=(h1r+h1i)*(h2r-h2i):  P_re = m1+m2, P_im = m3-m1+m2.
    # T = P_re^T A + P_im^T B  =  m1^T(A-B) + m2^T(A+B) + m3^T B,
    # where A = [WnRe|WnIm], B = [-WnIm|WnRe]. One trio per shift class,
    # restricted to the class's Y window.
    entries = [(np.concatenate([FmRe, FmIm], axis=1), M),
               (FnRe, M), (FnIm, M), ((-FnIm), M)]
    for kcls in sorted(prep["classes"]):
        Yk = prep["Y"][prep["classes"][kcls]]
        thw = 2 * np.pi * np.outer(k, Yk) / N
        WnRe = (np.cos(thw) / N).astype(np.float32)   # [128, nyc]
        WnIm = (np.sin(thw) / N).astype(np.float32)
        entries += [
            (np.concatenate([WnRe + WnIm, WnIm - WnRe], axis=1), M),  # A - B
            (np.concatenate([WnRe - WnIm, WnIm + WnRe], axis=1), M),  # A + B
            (np.concatenate([-WnIm, WnRe], axis=1), M),               # B
        ]
    entries += [(WmRe, UH), (WmImNeg, UH)]
    CPACK_W = sum(arr.shape[1] for arr, _ in entries)
    cpack = np.zeros((M, CPACK_W), np.float32)
    o = 0
    for arr, rows in entries:
        w = arr.shape[1]
        cpack[0:rows, o:o + w] = arr
        o += w
    return {"cpack": cpack.astype(BF16)}


def _build_program(prep, repeat=1):
    import concourse.bacc as bacc
    import concourse.mybir as mybir
    import concourse.tile as tile

    f32 = mybir.dt.float32
    bf16 = mybir.dt.bfloat16
    NX, NY = prep["NX"], prep["NY"]
    n_rows, nsub = prep["n_rows"], prep["nsub"]
    runs = prep["runs"]
    nmaps = J * C + nsub  # 32 m1-side + nsub m2-side maps
    ST = 2 * UH           # per-map hat stride: re|im planes interleaved

    # shift classes: per-row Y window widths
    rcls = prep["row_cls"]
    classes = prep["classes"]
    cls_list = sorted(classes)
    nyc = {kc: len(classes[kc]) for kc in cls_list}
    rows_of_cls = {kc: [i for i in range(n_rows) if rcls[i] == kc]
                   for kc in cls_list}
    seg_off = {}
    OUTW = 0
    for kc in cls_list:
        seg_off[kc] = OUTW
        OUTW += len(rows_of_cls[kc]) * nyc[kc]
    # stage-1 rows per PSUM group: sized so every class drains at a similar
    # row lag (bounds how long product tiles stay live in the ring)
    GTk = {kc: max(2, min(512 // (2 * nyc[kc]),
                          round(44 * len(rows_of_cls[kc]) / max(1, n_rows))))
           for kc in cls_list}
    # stage-2 group partition per class (rows per group, PSUM-bank bounded)
    bparts = {}
    for kc in cls_list:
        left = len(rows_of_cls[kc])
        gmax = max(1, 512 // nyc[kc])
        parts = []
        while left > 0:
            g = min(gmax, left)
            parts.append(g)
            left -= g
        bparts[kc] = parts

    nc = bacc.Bacc("TRN2", target_bir_lowering=False, debug=False,
                   num_devices=NCORES)

    def din(name, shape):
        return nc.dram_tensor(name, list(shape), bf16, kind="ExternalInput").ap()

    # host supplies maps pre-transposed to [m, z, n] so the load is contiguous
    xmaps = din("xmaps", (M, nmaps * N))
    CPACK_W = 2 * UH + 3 * M + sum(3 * 2 * nyc[kc] for kc in cls_list) + 2 * NX
    cpk = din("cpack", (M, CPACK_W))
    # out: per-class segments [x, (row, y-window)] bf16; host scatters back
    out = nc.dram_tensor("out", [NX, OUTW], bf16, kind="ExternalOutput").ap()

    with tile.TileContext(nc) as tc:
        with tc.tile_pool(name="const", bufs=1) as cpool:
            c_all = cpool.tile([M, CPACK_W], bf16)
            off = [0]

            def cslice(w, rows=M):
                o = off[0]
                off[0] += w
                return c_all[0:rows, o:o + w]

            c_Fm2 = cslice(2 * UH)
            c_FnRe, c_FnIm, c_FnImNeg = cslice(M), cslice(M), cslice(M)
            c_Wn = {kc: (cslice(2 * nyc[kc]), cslice(2 * nyc[kc]),
                         cslice(2 * nyc[kc])) for kc in cls_list}
            c_WmRe, c_WmImNeg = cslice(NX, UH), cslice(NX, UH)

            hat_ctx = tc.tile_pool(name="hatx", bufs=2 if repeat > 1 else 1)
            hat_pool = hat_ctx.__enter__()
            # interleaved [map, re|im, u] planes; allocated per repetition so
            # consecutive repetitions pipeline (rep k+1's FFT does not wait
            # for rep k's last product reads)
            hh = {}

            def alloc_hat():
                hat_t = hat_pool.tile([M, nmaps * ST], bf16, tag="hat")
                hs1_t = hat_pool.tile([M, J * C * UH], bf16, tag="hs1", bufs=1)
                hs2_t = hat_pool.tile([M, nsub * UH], bf16, tag="hs2", bufs=1)
                hh["hat"], hh["hs1"], hh["hs2"] = hat_t, hs1_t, hs2_t

            # FFT map groups: m2-side maps first so products unblock early.
            # Groups never straddle the m2/m1 boundary (hat slices stay
            # contiguous in the absolute map index).
            G6 = 6
            groups = [list(range(z0, min(z0 + G6, nmaps)))
                      for z0 in range(J * C, nmaps, G6)]
            groups += [list(range(z0, min(z0 + G6, J * C)))
                       for z0 in range(0, J * C, G6)]

            def hs_emit(zs, eng):
                # split at the m2/m1 boundary so each op is one plane kind
                parts = []
                cur = [zs[0]]
                for z in zs[1:]:
                    if (z >= J * C) == (cur[-1] >= J * C) and z == cur[-1] + 1:
                        cur.append(z)
                    else:
                        parts.append(cur)
                        cur = [z]
                parts.append(cur)
                for pzs in parts:
                    z0, z1 = pzs[0], pzs[-1] + 1
                    hv = hh["hat"][:, z0 * ST:z1 * ST].rearrange(
                        "p (z u) -> p z u", u=ST)
                    if z0 >= J * C:  # m2 side -> hs2 = re - im
                        o = hh["hs2"][:, (z0 - J * C) * UH:(z1 - J * C) * UH]
                        eng.tensor_sub(
                            o.rearrange("p (z u) -> p z u", u=UH),
                            hv[:, :, 0:UH], hv[:, :, UH:ST])
                    else:            # m1 side -> hs1 = re + im
                        o = hh["hs1"][:, z0 * UH:z1 * UH]
                        eng.tensor_add(
                            o.rearrange("p (z u) -> p z u", u=UH),
                            hv[:, :, 0:UH], hv[:, :, UH:ST])

            # runs partitioned by the FFT group holding their m1 map; the last
            # `npool` runs (tail rows, early-ready m1) go to GPSIMD, emitted
            # with the first m1 group
            nruns = len(runs)
            KPOOL = prep.get("npool", 0)
            gi_of_map = {}
            for gi, zs in enumerate(groups):
                for z in zs:
                    gi_of_map[z] = gi
            first_m1_gi = gi_of_map[0]
            runs_of_group = {gi: [] for gi in range(len(groups))}
            for ri, (a, s0, R) in enumerate(runs[:nruns - KPOOL]):
                runs_of_group[gi_of_map[a]].append(ri)
            runs_of_group[first_m1_gi] = (
                list(range(nruns - KPOOL, nruns)) + runs_of_group[first_m1_gi])

            with tc.tile_pool(name="fftsb", bufs=6) as fsb, \
                 tc.tile_pool(name="fftsA", bufs=3) as fsA, \
                 tc.tile_pool(name="fpsA", bufs=2, space="PSUM") as fpsA, \
                 tc.tile_pool(name="fpsB", bufs=2, space="PSUM") as fpsB, \
                 tc.tile_pool(name="tt", bufs=9) as tpool, \
                 tc.tile_pool(name="ttp", bufs=max(1, KPOOL)) as ppool, \
                 tc.tile_pool(name="tsb", bufs=2) as tsbp, \
                 tc.tile_pool(name="stg", bufs=2) as stgp, \
                 tc.tile_pool(name="psT", bufs=2, space="PSUM") as psT, \
                 tc.tile_pool(name="psO", bufs=2, space="PSUM") as psO:
                # input DMA: 8-map slabs, m2-side first
                xt_tiles = {}
                first = True
                dma_order = list(range(J * C, nmaps)) + list(range(0, J * C))
                for h0 in range(0, len(dma_order), 8):
                    hz = dma_order[h0:h0 + 8]
                    xt_t = fsb.tile([M, len(hz) * N], bf16, tag=f"xb{hz[0]}")
                    nc.sync.dma_start(
                        xt_t[:], xmaps[:, hz[0] * N:(hz[-1] + 1) * N])
                    if first:  # consts land right behind the first maps
                        nc.sync.dma_start(c_all[:], cpk[:])
                        first = False
                    for k, z in enumerate(hz):
                        xt_tiles[z] = xt_t[:, k * N:(k + 1) * N]

                ci = [0]

                def fft_group(gi, zs):
                    # early groups: DVE is idle until products start, so copies
                    # alternate DVE/ACT; later: ACT only
                    def fcopy(dst, src):
                        if gi < 2 and ci[0] % 2 == 0:
                            nc.vector.tensor_copy(dst, src)
                        else:
                            nc.scalar.copy(dst, src)
                        ci[0] += 1

                    ng = len(zs)
                    sA = fsA.tile([M, ng * ST], bf16, tag="sA")
                    for b0 in range(0, ng, 3):
                        bz = zs[b0:b0 + 3]
                        nb = len(bz)
                        # T1: A[n, (z, re|im u)] = x_z^T @ [FmRe|FmIm]
                        pA = fpsA.tile([M, nb * ST], f32, tag="pA")
                        for k, z in enumerate(bz):
                            nc.tensor.matmul(pA[:, k * ST:(k + 1) * ST],
                                             xt_tiles[z], c_Fm2,
                                             start=True, stop=True)
                        fcopy(sA[:, b0 * ST:(b0 + nb) * ST], pA[:])
                    # T2: hat[v, (z, u)] = Fn^T @ A, batched over the group
                    sA3 = sA.rearrange("p (z u) -> p z u", u=ST)
                    a_re, a_im = sA3[:, :, 0:UH], sA3[:, :, UH:ST]
                    hv = hh["hat"][:, zs[0] * ST:(zs[-1] + 1) * ST].rearrange(
                        "p (z u) -> p z u", u=ST)
                    pRe = fpsB.tile([M, ng * UH], f32, tag="pB")
                    pRe3 = pRe[:].rearrange("p (z u) -> p z u", u=UH)
                    nc.tensor.matmul(pRe3, c_FnRe, a_re, start=True, stop=False)
                    nc.tensor.matmul(pRe3, c_FnImNeg, a_im, start=False, stop=True)
                    fcopy(hv[:, :, 0:UH], pRe3)
                    pIm = fpsB.tile([M, ng * UH], f32, tag="pB")
                    pIm3 = pIm[:].rearrange("p (z u) -> p z u", u=UH)
                    nc.tensor.matmul(pIm3, c_FnRe, a_im, start=True, stop=False)
                    nc.tensor.matmul(pIm3, c_FnIm, a_re, start=False, stop=True)
                    fcopy(hv[:, :, UH:ST], pIm3)

                row_t = [None] * (n_rows * repeat)
                row_starts = np.concatenate(
                    [[0], np.cumsum([R for _, _, R in runs])]).astype(int)

                def products(ri, rep):
                    a, s0, R = runs[ri]
                    on_pool = ri >= nruns - KPOOL
                    eng = nc.gpsimd if on_pool else nc.vector
                    tp = ppool if on_pool else tpool
                    t_m1 = tp.tile([M, R * UH], bf16, tag="t_m1")
                    t_m2 = tp.tile([M, R * UH], bf16, tag="t_m2")
                    t_m3 = tp.tile([M, R * UH], bf16, tag="t_m3")
                    hat, hs1, hs2 = hh["hat"], hh["hs1"], hh["hs2"]
                    a_re = hat[:, a * ST:a * ST + UH].unsqueeze(1).broadcast_to([M, R, UH])
                    a_im = hat[:, a * ST + UH:(a + 1) * ST].unsqueeze(1).broadcast_to([M, R, UH])
                    a_s = hs1[:, a * UH:(a + 1) * UH].unsqueeze(1).broadcast_to([M, R, UH])
                    b_ar = hat[:, (J * C + s0) * ST:(J * C + s0 + R) * ST].rearrange(
                        "p (r u) -> p r u", u=ST)
                    b_s = hs2[:, s0 * UH:(s0 + R) * UH].rearrange("p (r u) -> p r u", r=R)
                    eng.tensor_mul(t_m1[:].rearrange("p (r u) -> p r u", r=R),
                                   a_re, b_ar[:, :, 0:UH])
                    eng.tensor_mul(t_m2[:].rearrange("p (r u) -> p r u", r=R),
                                   a_im, b_ar[:, :, UH:ST])
                    eng.tensor_mul(t_m3[:].rearrange("p (r u) -> p r u", r=R),
                                   a_s, b_s)
                    r0 = rep * n_rows + row_starts[ri]
                    for i in range(R):
                        row_t[r0 + i] = (
                            t_m1[:, i * UH:(i + 1) * UH],
                            t_m2[:, i * UH:(i + 1) * UH],
                            t_m3[:, i * UH:(i + 1) * UH],
                        )

                # per-class stage-1/2 state, reset each repetition
                cstate = {}

                def reset_cstate():
                    for kc in cls_list:
                        cstate[kc] = dict(pend=[], bparts=list(bparts[kc]),
                                          sT=None, fill=0, cap=0, opos=0)

                def s2_fire(kc):
                    st = cstate[kc]
                    w2 = 2 * nyc[kc]
                    g = st["cap"]
                    T3 = st["sT"].rearrange("p (r w) -> p r w", w=w2)
                    pO = psO.tile([NX, g * nyc[kc]], f32, tag="pO")
                    pO3 = pO[:].rearrange("p (r y) -> p r y", y=nyc[kc])
                    nc.tensor.matmul(pO3, c_WmRe, T3[:, :, 0:nyc[kc]],
                                     start=True, stop=False)
                    nc.tensor.matmul(pO3, c_WmImNeg, T3[:, :, nyc[kc]:w2],
                                     start=False, stop=True)
                    stg = stgp.tile([NX, g * nyc[kc]], bf16, tag="stg")
                    nc.scalar.copy(stg[:], pO[:])
                    col = seg_off[kc] + st["opos"] * nyc[kc]
                    nc.sync.dma_start(out[:, col:col + g * nyc[kc]], stg[:])
                    st["opos"] += g
                    st["sT"] = None

                def s1_fire(kc, rows):
                    st = cstate[kc]
                    w2 = 2 * nyc[kc]
                    g = len(rows)
                    wn1, wn2, wn3 = c_Wn[kc]
                    pT1 = psT.tile([UH, g * w2], f32, tag="pT1")
                    for i, r in enumerate(rows):
                        tm1, tm2, tm3 = row_t[r]
                        o = pT1[:, i * w2:(i + 1) * w2]
                        nc.tensor.matmul(o, tm1, wn1, start=True, stop=False)
                        nc.tensor.matmul(o, tm2, wn2, start=False, stop=False)
                        nc.tensor.matmul(o, tm3, wn3, start=False, stop=True)
                    if st["sT"] is None:
                        st["cap"] = st["bparts"].pop(0)
                        st["fill"] = 0
                        sT_new = tsbp.tile([UH, st["cap"] * w2], bf16,
                                           tag=f"sT{kc}")
                        st["sT"] = sT_new
                    o0 = st["fill"]
                    nc.scalar.copy(st["sT"][:, o0 * w2:(o0 + g) * w2], pT1[:])
                    st["fill"] += g
                    if st["fill"] >= st["cap"]:
                        s2_fire(kc)

                def drain(kc, final):
                    st = cstate[kc]
                    while st["pend"]:
                        room = (st["cap"] - st["fill"]) if st["sT"] is not None \
                            else (st["bparts"][0] if st["bparts"] else 0)
                        if room <= 0:
                            break
                        full = min(GTk[kc], room)
                        if len(st["pend"]) < full and not final:
                            break
                        take = min(full, len(st["pend"]))
                        s1_fire(kc, st["pend"][:take])
                        del st["pend"][:take]

                sw = [0]  # rows pushed to class queues

                def emit_groups(rows_done):
                    while sw[0] < rows_done:
                        r = sw[0]
                        kc = int(rcls[r % n_rows])
                        cstate[kc]["pend"].append(r)
                        sw[0] += 1
                        drain(kc, final=False)

                # ---- interleaved pipeline: FFT group k+1 is emitted before
                # the products of group k so each engine's in-order queue
                # keeps the FFT one group ahead of the product stream ----
                for rep in range(repeat):
                    alloc_hat()
                    reset_cstate()
                    fft_group(0, groups[0])
                    hs_emit(groups[0], nc.vector)
                    fft_group(1, groups[1])
                    hs_emit(groups[1], nc.vector)
                    for k in range(1, len(groups)):
                        if k + 1 < len(groups):
                            fft_group(k + 1, groups[k + 1])
                        for ri in runs_of_group.get(k, []):
                            products(ri, rep)
                        if k + 1 < len(groups):
                            hs_emit(groups[k + 1], nc.vector)
                        nonpool = [ri for ri in runs_of_group.get(k, [])
                                   if ri < nruns - KPOOL]
                        done = rep * n_rows + (row_starts[nonpool[-1] + 1]
                                               if nonpool else 0)
                        emit_groups(done)
                    # flush this repetition (pool-run rows + partial groups)
                    emit_groups((rep + 1) * n_rows)
                    for kc in cls_list:
                        drain(kc, final=True)
            hat_ctx.__exit__(None, None, None)

    nc.compile()
    return nc


def _fallback(xpsi, masks_shift, la1, la2, shifted, union_idx):
    hatx = np.fft.fft2(xpsi.astype(np.float64))
    h1 = hatx[la1[:, 0], :, la1[:, 1]]
    h2 = hatx[la2[:, 0], :, la2[:, 1]]
    corr = np.fft.ifft2(h1 * np.conj(h2)).real
    masked = corr * masks_shift[shifted][:, None]
    Pm, Bb, Mm, Nn = masked.shape
    return masked.reshape(Pm, Bb, Mm * Nn)[:, :, union_idx].astype(np.float32)


def kernel(**inputs):
    xpsi = np.ascontiguousarray(np.asarray(inputs["xpsi"], dtype=np.float32))
    masks_shift = np.asarray(inputs["masks_shift"], dtype=np.float32)
    la1 = np.asarray(inputs["la1"], dtype=np.int64)
    la2 = np.asarray(inputs["la2"], dtype=np.int64)
    shifted = np.asarray(inputs["shifted"], dtype=np.int64)
    union_idx = np.asarray(inputs["union_idx"], dtype=np.int64)

    if xpsi.shape != (J, B, C, M, N) or (shifted < 0).any() or \
            (shifted >= masks_shift.shape[0]).any():
        return _fallback(xpsi, masks_shift, la1, la2, shifted, union_idx)
    prep = _host_prep(la1, la2, shifted, union_idx, masks_shift)
    if prep is None:
        return _fallback(xpsi, masks_shift, la1, la2, shifted, union_idx)

    try:
        return _run_device(xpsi, masks_shift, la1, shifted, union_idx, prep)
    except Exception:
        return _fallback(xpsi, masks_shift, la1, la2, shifted, union_idx)


def _in_maps(xpsi, prep):
    """Per-core device input dicts (bf16)."""
    cst = _consts(prep)
    xflat = xpsi.transpose(0, 2, 1, 3, 4).reshape(J * C, B, M, N)
    in_maps = []
    for core in range(NCORES):
        b, parity = divmod(core, 2)
        sub = prep["sub_e"] if parity == 0 else prep["sub_o"]
        xm = np.concatenate([xflat[:, b], xflat[sub, b]], axis=0)
        xm = xm.transpose(1, 0, 2).reshape(M, -1).astype(BF16)  # [m, z*n]
        m = {"xmaps": np.ascontiguousarray(xm)}
        m.update(cst)
        in_maps.append(m)
    return in_maps


def _host_mask(masks_shift, shifted, prep):
    """[n_rows, NX*NY] x-major mask values in device row order."""
    X, Y = prep["X"], prep["Y"]
    p_sorted_even = prep["pe"][prep["order"]]
    mk = masks_shift[shifted[p_sorted_even]]        # [n_rows, 128, 128]
    mv = mk[:, X[:, None], Y[None, :]]              # [n_rows, NX, NY]
    return mv.reshape(prep["n_rows"], -1)


def _cache_key(prep):
    return (prep["NX"], prep["NY"], prep["n_rows"], tuple(prep["runs"]),
            tuple(prep["row_cls"].tolist()),
            tuple((k, tuple(v.tolist())) for k, v in
                  sorted(prep["classes"].items())))


def _run_device(xpsi, masks_shift, la1, shifted, union_idx, prep):
    key = _cache_key(prep)
    if key not in _CACHE:
        _CACHE[key] = _build_program(prep)
    nc = _CACHE[key]

    NX, NY = prep["NX"], prep["NY"]
    order, n_rows = prep["order"], prep["n_rows"]
    in_maps = _in_maps(xpsi, prep)

    from concourse.bass_utils import run_bass_kernel_spmd
    res = run_bass_kernel_spmd(nc, in_maps, list(range(NCORES)))

    P = la1.shape[0]
    rcls, classes = prep["row_cls"], prep["classes"]
    cls_list = sorted(classes)
    mflat = _host_mask(masks_shift, shifted, prep)  # [n_rows, NX*NY]
    out = np.empty((P, B, NX * NY), np.float32)
    inv = np.empty(n_rows, np.int64)
    inv[order] = np.arange(n_rows)                  # row of sorted order for pe[k]
    for core in range(NCORES):
        b, parity = divmod(core, 2)
        dev = np.asarray(res.results[core]["out"], dtype=np.float32)
        # device out: per-class segments [NX, rows_k, nyc_k]; scatter the
        # Y-window columns back into the full union grid (rest stays zero)
        full = np.zeros((n_rows, NX, NY), np.float32)
        off = 0
        for kc in cls_list:
            rk = np.where(rcls == kc)[0]
            Yi = classes[kc]
            nyk = len(Yi)
            seg = dev[:, off:off + len(rk) * nyk].reshape(NX, len(rk), nyk)
            sub = np.zeros((len(rk), NX, NY), np.float32)
            sub[:, :, Yi] = seg.transpose(1, 0, 2)
            full[rk] = sub
            off += len(rk) * nyk
        flat = full.reshape(n_rows, NX * NY) * mflat
        p_idx = prep["pe"] if parity == 0 else prep["po"]
        out[p_idx, b, :] = flat[inv]
    return out


if __name__ == "__main__":
    import importlib
    ref = importlib.import_module("reference")
    import jax
    cpu = jax.devices("cpu")[0]
    with jax.default_device(cpu):
        raw = ref.setup_inputs()
        ins = {k: np.asarray(v) for k, v in raw.items()}
        exp = np.asarray(ref.reference(**{k: jax.device_put(v, cpu) for k, v in raw.items()}))
    got = kernel(**ins)
    d = np.linalg.norm(got - exp) / np.linalg.norm(exp)
    print("rel:", d, "maxabs:", np.abs(got - exp).max())
